# revision 25
# baseline (speedup 1.0000x reference)
"""Trainium2 Bass kernel for nn_AttnCalc (coverage attention).

Contract: kernel(**inputs) takes FULL unsharded numpy inputs, distributes
batch-parallel across 8 NeuronCores, returns the full
(context_vector, attn_weights, new_coverage) tuple like the reference.

Math per batch b:
  enc_feat = enc[b] @ attn_w.T + attn_b          [L,H]
  dec_feat = dec_w @ hidden[b] + dec_b           [H]
  cov_feat = w_eff @ coverage[b] + cvg_b         [L]   (w_eff = cvg_w[:,:,0,(H-1)//2])
  feats    = tanh(enc_feat + dec_feat + cov_feat[:,None])
  scores   = feats @ v[b]  (masked, softmax over L) -> aw
  new_cov  = coverage[b] + aw
  context  = aw @ enc[b]                         [H]

Device layout (per core, BLOC=8 batches):
  [*,L]-major work uses tiles [128 (H-chunk), L] (transposed layout,
  contraction over H on PE partitions).  cov_feat and dec_feat(+biases)
  are folded into the enc_feat PSUM accumulation (K=1 rank-1 matmuls)
  and the tanh activation bias.  The context contraction (over L) runs
  on the Vector engine as tensor_tensor_reduce against a DMA-broadcast
  of aw, so encoder_outputs is only loaded once, in transposed form.

The target walrus build allows only ONE semaphore wait per TPB compute
instruction, so the program is arranged so every compute op has at most
one cross-engine dependency; tiny "absorber" ops (1x1 matmul / copy)
pick up the remaining waits ahead of real work.
"""

import sys
import os

sys.path.insert(0, "/opt/trn_rl_repo")

import numpy as np

import concourse.bass as bass
import concourse.tile as tile
from concourse import mybir
from concourse.bass_utils import run_bass_kernel_spmd
from concourse.tile_rust import add_dep_helper

B, L, H = 64, 512, 512
NCORES = 8
BLOC = B // NCORES          # batches per core
P = 128                     # SBUF partitions
PC = H // P                 # 128-chunks along H (== along L)
F32 = mybir.dt.float32
Tanh = mybir.ActivationFunctionType.Tanh
Exp = mybir.ActivationFunctionType.Exp
Copy = mybir.ActivationFunctionType.Copy

_CACHE = {}


def _build_program():
    nc = bass.Bass()

    encT = nc.declare_dram_parameter("encT", [BLOC, H, L], F32, isOutput=False)
    attn_wT = nc.declare_dram_parameter("attn_wT", [H, H], F32, isOutput=False)
    w_effT = nc.declare_dram_parameter("w_effT", [L, L], F32, isOutput=False)
    dec_wT = nc.declare_dram_parameter("dec_wT", [H, H], F32, isOutput=False)
    vT = nc.declare_dram_parameter("vT", [H, BLOC], F32, isOutput=False)
    covT = nc.declare_dram_parameter("covT", [L, BLOC], F32, isOutput=False)
    hidT = nc.declare_dram_parameter("hidT", [H, BLOC], F32, isOutput=False)
    cov_in = nc.declare_dram_parameter("cov_in", [BLOC, L], F32, isOutput=False)
    maskb = nc.declare_dram_parameter("maskb", [BLOC, L], F32, isOutput=False)
    attn_b_r = nc.declare_dram_parameter("attn_b_r", [1, H], F32, isOutput=False)
    dec_b_r = nc.declare_dram_parameter("dec_b_r", [1, H], F32, isOutput=False)
    cvg_b_r = nc.declare_dram_parameter("cvg_b_r", [1, L], F32, isOutput=False)

    aw_out = nc.declare_dram_parameter("aw_out", [BLOC, L], F32, isOutput=True)
    ncov_out = nc.declare_dram_parameter("ncov_out", [BLOC, L], F32, isOutput=True)
    ctxT_out = nc.declare_dram_parameter("ctxT_out", [P, PC, BLOC], F32, isOutput=True)

    def row3(dram2d):
        # [BLOC, L] dram -> [1, BLOC, L] AP so rows can live on partition 0
        return dram2d[:, :].rearrange("b l -> (b l)")[None].rearrange(
            "o (b l) -> o b l", b=BLOC)

    with tile.TileContext(nc) as tc:
        with (
            tc.tile_pool(name="const", bufs=1) as const,
            tc.tile_pool(name="enc", bufs=2) as epool,
            tc.tile_pool(name="feat", bufs=2) as fpool,
            tc.tile_pool(name="awb", bufs=2) as bpool,
            tc.tile_pool(name="prod", bufs=2) as prpool,
            tc.tile_pool(name="eps", bufs=4, space=bass.MemorySpace.PSUM) as ppool,
            tc.tile_pool(name="scps", bufs=2, space=bass.MemorySpace.PSUM) as scpool,
            tc.tile_pool(name="setps", bufs=1, space=bass.MemorySpace.PSUM) as stpool,
            tc.tile_pool(name="dumps", bufs=1, space=bass.MemorySpace.PSUM) as dumpool,
            tc.tile_pool(name="dram", bufs=1, space=bass.MemorySpace.DRAM) as dpool,
        ):
            # -------- wait absorbers (1x1 ops that pick up semaphore waits
            # so real compute ops never need more than one) --------
            dum_t = dumpool.tile([1, 64], F32, tag="dummy")
            dve_dum = const.tile([1, 256], F32)
            act_dum = const.tile([1, 256], F32)
            _ctr = {"pe": 0, "dve": 0, "act": 0}

            def pe_abs(ap):
                i = _ctr["pe"] = (_ctr["pe"] + 1) % 64
                return nc.tensor.matmul(dum_t[0:1, i:i + 1], ap, ap,
                                        start=True, stop=True)

            def dve_abs(ap):
                i = _ctr["dve"] = (_ctr["dve"] + 1) % 256
                return nc.vector.tensor_copy(dve_dum[0:1, i:i + 1], ap)

            def act_abs(ap):
                i = _ctr["act"] = (_ctr["act"] + 1) % 256
                return nc.scalar.activation(act_dum[0:1, i:i + 1], ap, Copy)

            def pin(real, *deps):
                for d in deps:
                    add_dep_helper(real.ins, d.ins, sync=False,
                                   reason="absorber ordering")

            # ---------------- constants ----------------
            wA = const.tile([P, PC, H], F32)   # attn_wT  [h=k*128+p][o]
            wC = const.tile([P, PC, L], F32)   # w_effT   [l'=k*128+p][l]
            wD = const.tile([P, PC, H], F32)   # dec_wT   [h=k*128+p][o]
            vS = const.tile([P, PC, BLOC], F32)
            cS = const.tile([P, PC, BLOC], F32)
            hS = const.tile([P, PC, BLOC], F32)
            nc.sync.dma_start(out=wA, in_=attn_wT[:, :].rearrange("(k p) o -> p k o", p=P))
            nc.sync.dma_start(out=wC, in_=w_effT[:, :].rearrange("(k p) o -> p k o", p=P))
            nc.sync.dma_start(out=wD, in_=dec_wT[:, :].rearrange("(k p) o -> p k o", p=P))
            nc.sync.dma_start(out=vS, in_=vT[:, :].rearrange("(k p) b -> p k b", p=P))
            nc.sync.dma_start(out=cS, in_=covT[:, :].rearrange("(k p) b -> p k b", p=P))
            nc.sync.dma_start(out=hS, in_=hidT[:, :].rearrange("(k p) b -> p k b", p=P))

            ab_row = const.tile([1, H], F32)
            db_row = const.tile([1, H], F32)
            cb_row = const.tile([1, L], F32)
            nc.sync.dma_start(out=ab_row, in_=attn_b_r[:, :])
            nc.sync.dma_start(out=db_row, in_=dec_b_r[:, :])
            nc.sync.dma_start(out=cb_row, in_=cvg_b_r[:, :])

            ones_b = const.tile([1, BLOC], F32)
            nc.vector.memset(ones_b, 1.0)
            ones_p = const.tile([1, P], F32)
            nc.vector.memset(ones_p, 1.0)
            d_ab = dve_abs(ab_row[0:1, 0:1])
            d_db = dve_abs(db_row[0:1, 0:1])
            brow = const.tile([1, H], F32)     # attn_b + dec_b
            badd = nc.vector.tensor_add(brow, ab_row, db_row)
            pin(badd, d_ab, d_db)

            # per-batch rows as disjoint slices of persistent [1,BLOC,L]
            # tiles on partition 0 (no slot reuse -> no release waits)
            mb = const.tile([1, BLOC, L], F32)
            covin = const.tile([1, BLOC, L], F32)
            nc.sync.dma_start(out=mb, in_=row3(maskb))
            nc.sync.dma_start(out=covin, in_=row3(cov_in))
            d_mb = dve_abs(mb[0:1, 0, 0:1])
            d_cvn = dve_abs(covin[0:1, 0, 0:1])

            sc = const.tile([1, BLOC, L], F32)      # scores -> exp -> aw
            nmx = const.tile([1, BLOC, 1], F32)
            se = const.tile([1, BLOC, 1], F32)
            rse = const.tile([1, BLOC, 1], F32)
            cov_sb = const.tile([1, BLOC, L], F32)  # cov_feat rows
            ctx_all = const.tile([P, PC, BLOC], F32)

            # ---------------- setup: cov_feat rows ----------------
            d_wC = pe_abs(wC[0:1, 0, 0:1])
            d_cS = pe_abs(cS[0:1, 0, 0:1])
            d_cb = pe_abs(cb_row[0:1, 0:1])
            d_on = pe_abs(ones_b[0:1, 0:1])
            cov_ps = stpool.tile([BLOC, L], F32, tag="setup")
            for k in range(PC):
                mm = nc.tensor.matmul(cov_ps, cS[:, k, :], wC[:, k, :],
                                      start=(k == 0), stop=False)
                if k == 0:
                    pin(mm, d_wC, d_cS)
            mm = nc.tensor.matmul(cov_ps, ones_b[:, :], cb_row[:, :],
                                  start=False, stop=True)
            pin(mm, d_cb, d_on)
            # bounce through DRAM to turn [BLOC(part), L] into row-major
            # [1, BLOC, L] on partition 0
            cov_tmp = const.tile([BLOC, L], F32)
            nc.vector.tensor_copy(cov_tmp, cov_ps)
            cov_dram = dpool.tile([BLOC, L], F32)
            nc.sync.dma_start(out=cov_dram[:, :], in_=cov_tmp)
            nc.sync.dma_start(out=cov_sb, in_=row3(cov_dram))
            d_cov = pe_abs(cov_sb[0:1, 0, 0:1])

            # ---------------- setup: tanh bias rows ----------------
            # bias_sb[p, o_chunk, b] = (dec_w @ hidden[b] + dec_b + attn_b)[o]
            d_wD = pe_abs(wD[0:1, 0, 0:1])
            d_hS = pe_abs(hS[0:1, 0, 0:1])
            d_br = pe_abs(brow[0:1, 0:1])
            dps = stpool.tile([P, PC, BLOC], F32, tag="setup")
            for o in range(PC):
                for k in range(PC):
                    mm = nc.tensor.matmul(dps[:, o, :], wD[:, k, o * P:(o + 1) * P],
                                          hS[:, k, :], start=(k == 0), stop=False)
                    if k == 0:
                        pin(mm, d_wD, d_hS)
                bmm = nc.tensor.matmul(dps[:, o, :], brow[0:1, o * P:(o + 1) * P],
                                       ones_b[:, :], start=False, stop=True)
                pin(bmm, d_br)
            bias_sb = const.tile([P, PC, BLOC], F32)
            nc.scalar.copy(bias_sb, dps)
            a_bias = act_abs(bias_sb[0:1, 0, 0:1])
            d_vS = pe_abs(vS[0:1, 0, 0:1])
            d_wA = pe_abs(wA[0:1, 0, 0:1])
            d_op = pe_abs(ones_p[0:1, 0:1])

            # ---------------- main per-batch pipeline ----------------
            prev_exp = None
            prev_df = None
            for b in range(BLOC):
                # wait-free SP-stream slots for the legalizer, kept near
                # this batch's DMAs by nosync pins
                sps = [nc.sync.nop(nofuse=True) for _ in range(3)]
                if prev_df is not None:
                    pin(sps[0], prev_df)
                pin(sps[1], sps[0])
                pin(sps[2], sps[1])
                eT = epool.tile([P, PC, L], F32)
                eT_dma = nc.sync.dma_start(
                    out=eT, in_=encT[b].rearrange("(k p) l -> p k l", p=P))
                pin(eT_dma, sps[2])
                d_e = pe_abs(eT[0:1, 0, 0:1])       # eT DMA wait -> PE
                v_e = dve_abs(eT[0:1, 0, 0:1])      # eT DMA wait -> DVE

                # enc_featT[o,l] + cov_feat[l] in PSUM; tanh(+bias) -> ft
                # a_slot is a wait-free ACT op the legalizer can move
                # bounced same-engine waits onto; anchor it after the
                # previous batch's exp so it lands between batches
                a_slot = act_abs(ones_b[0:1, 0:1])
                if prev_exp is not None:
                    pin(a_slot, prev_exp)
                ft = fpool.tile([P, PC, L], F32)
                first_th = None
                for o in range(PC):
                    ps = ppool.tile([P, L], F32, tag="encps")
                    for k in range(PC):
                        mm = nc.tensor.matmul(ps, wA[:, k, o * P:(o + 1) * P],
                                              eT[:, k, :], start=(k == 0),
                                              stop=False)
                        if k == 0:
                            pin(mm, d_e)
                            if b == 0:
                                pin(mm, d_wA)
                    cmm = nc.tensor.matmul(ps, ones_p[:, :],
                                           cov_sb[0:1, b, :],
                                           start=False, stop=True)
                    if b == 0:
                        pin(cmm, d_cov, d_op)
                    th = nc.scalar.activation(
                        out=ft[:, o, :], in_=ps, func=Tanh,
                        bias=bias_sb[:, o, b:b + 1], scale=1.0)
                    if first_th is None:
                        first_th = th
                        pin(th, a_slot)
                    if b == 0 and o == 0:
                        pin(th, a_bias)

                # scores[l] = sum_o feats[o,l] * v[b,o]
                d_f = pe_abs(ft[0:1, 0, 0:1])
                prev_df = d_f
                sc_ps = scpool.tile([1, L], F32)
                for k in range(PC):
                    mm = nc.tensor.matmul(sc_ps, vS[:, k, b:b + 1], ft[:, k, :],
                                          start=(k == 0), stop=(k == 3))
                    if k == 0:
                        pin(mm, d_f)
                        if b == 0:
                            pin(mm, d_vS)

                # masked softmax over l, in sc row b (aw ends up there)
                scr = sc[0:1, b, :]
                madd = nc.vector.tensor_add(scr, sc_ps, mb[0:1, b, :])
                if b == 0:
                    pin(madd, d_mb)
                nc.vector.tensor_reduce(out=nmx[0:1, b, :], in_=scr,
                                        axis=mybir.AxisListType.X,
                                        op=mybir.AluOpType.max, negate=True)
                prev_exp = nc.scalar.activation(out=scr, in_=scr, func=Exp,
                                                bias=nmx[0:1, b, :], scale=1.0,
                                                accum_out=se[0:1, b, :])
                nc.vector.reciprocal(rse[0:1, b, :], se[0:1, b, :])
                nc.vector.tensor_scalar_mul(scr, scr, rse[0:1, b, :])

                # aw row out + broadcast back for the context reduction
                sp_aw = nc.sync.nop(nofuse=True)
                pin(sp_aw, mm)           # anchor inside this batch
                aw_dma = nc.sync.dma_start(out=aw_out[b:b + 1, :], in_=scr)
                pin(aw_dma, sp_aw)
                gp_slots = [nc.gpsimd.nop(nofuse=True) for _ in range(2)]
                pin(gp_slots[0], mm)
                pin(gp_slots[1], gp_slots[0])
                aw_b = bpool.tile([P, L], F32, tag="awb")
                bc_dma = nc.gpsimd.dma_start(
                    out=aw_b, in_=aw_out[b:b + 1, :].to_broadcast([P, L]))
                pin(bc_dma, gp_slots[1])

                # new_coverage row (in place over covin row)
                ncadd = nc.vector.tensor_add(covin[0:1, b, :],
                                              covin[0:1, b, :], scr)
                if b == 0:
                    pin(ncadd, d_cvn)
                sp_nc = nc.sync.nop(nofuse=True)
                pin(sp_nc, aw_dma)
                nc_dma = nc.sync.dma_start(out=ncov_out[b:b + 1, :],
                                           in_=covin[0:1, b, :])
                pin(nc_dma, sp_nc)

                # context[h] = sum_l aw[l] * encT[h,l] on DVE
                # (tensor_tensor_reduce is a raw-ISA op this walrus cannot
                # encode, so use mul + reduce_sum)
                for o in range(PC):
                    prod = prpool.tile([P, L], F32, tag="prod")
                    tm = nc.vector.tensor_mul(prod, eT[:, o, :], aw_b)
                    if o == 0:
                        pin(tm, v_e)
                    nc.vector.reduce_sum(out=ctx_all[:, o, b:b + 1], in_=prod,
                                         axis=mybir.AxisListType.X)

            ctx_dma = nc.sync.dma_start(out=ctxT_out[:, :, :], in_=ctx_all)

            # tail landing slots: the framework kernel-tail drain waits on
            # every engine/queue semaphore at once; give the legalizer SP
            # instructions to spread those waits over
            tail = ctx_dma
            for _ in range(22):
                n = nc.sync.nop(nofuse=True)
                pin(n, tail)
                tail = n

    _legalize_waits(nc)
    return nc


# The nix walrus build (setupSyncWait) accepts only ONE sync wait per TPB
# instruction (compute and DMA alike).  Tile can emit several.  Because the
# committed instruction order is a topological order of the dependency
# graph, a wait whose producing semaphore update completes at block index p
# can be safely carried by ANY same-engine instruction at index > p that
# precedes the original carrier: engines execute in order, so the original
# instruction still starts after the wait is satisfied, and the producer
# (committed before the new carrier) cannot depend on it -- no deadlock.
# Assign waits to instructions as an interval matching problem.
def _legalize_waits(nc):
    import concourse.mybir as _mb

    fn = nc.m.functions[0]
    stuck = []
    NO_LANDING = ("InstISA", "InstEventSemaphore", "InstUnconditionalBranch",
                  "InstCall", "InstRegisterMove", "InstHalt")
    # one global stream per engine, blocks concatenated in order (engines
    # branch from block to block in order, so per-engine execution order is
    # block order)
    insts = []
    for blk in fn.blocks:
        insts.extend(blk.instructions)

    sem_hist = {}
    cum = {}
    streams = {}
    for i, inst in enumerate(insts):
        si = inst.sync_info
        if si is not None:
            for u in si.on_update:
                cum[u.id] = cum.get(u.id, 0) + u.update_value
                sem_hist.setdefault(u.id, []).append((i, cum[u.id]))
        streams.setdefault(inst.engine, []).append(i)

    def producer_idx(w):
        hist = sem_hist.get(w.id)
        if hist is None:
            return None            # unknown semaphore: not movable
        for i, v in hist:
            if v >= w.wait_value:
                return i
        return None

    for eng, stream in streams.items():
        movable_spos = []
        pinned = {}                # spos -> unmovable waits
        waits = []                 # (carrier_spos, producer_bidx, wait)
        has_multi = False
        for spos, i in enumerate(stream):
            inst = insts[i]
            si = inst.sync_info
            ws = list(si.on_wait) if si is not None else []
            if len(ws) > 1:
                has_multi = True

            def mov(w):
                if w.wait_reg is not None or w.wait_value <= 0:
                    return False
                p = producer_idx(w)
                return p is not None and p < i
            special = inst.__class__.__name__ in NO_LANDING
            unmov = [w for w in ws if special or not mov(w)]
            if unmov:
                pinned[spos] = unmov
            elif not special:
                movable_spos.append(spos)
            if special:
                continue
            best = {}
            for w in ws:
                if not mov(w):
                    continue
                if w.id not in best or w.wait_value > best[w.id].wait_value:
                    best[w.id] = w
            for w in best.values():
                waits.append((spos, producer_idx(w), w))
        if not has_multi:
            continue
        bidx_of = {spos: stream[spos] for spos in range(len(stream))}
        free = sorted(movable_spos)
        assign = {}
        for carrier, pbidx, w in sorted(waits, key=lambda t: (t[0], -t[1])):
            chosen = None
            for spos in reversed(free):
                if spos > carrier:
                    continue
                if bidx_of[spos] <= pbidx:
                    break
                chosen = spos
                break
            if chosen is None:
                stuck.append((insts[stream[carrier]].name,
                              insts[stream[carrier]].__class__.__name__,
                              w.ant_name, w.wait_value))
                continue
            free.remove(chosen)
            assign.setdefault(chosen, []).append(w)
        for spos in range(len(stream)):
            inst = insts[stream[spos]]
            si = inst.sync_info
            ups = list(si.on_update) if si is not None else []
            new_w = pinned.get(spos, []) + assign.get(spos, [])
            if si is None and not new_w:
                continue
            inst.sync_info = _mb.SyncInfo(on_wait=new_w, on_update=ups)
    if stuck:
        raise RuntimeError(f"wait legalization failed: {stuck[:8]}")


def _get_program():
    if "nc" not in _CACHE:
        _CACHE["nc"] = _build_program()
    return _CACHE["nc"]


def _prep_core_inputs(c, enc, maskf, hidden, coverage, attn_w, attn_b,
                      dec_w, dec_b, w_eff, cvg_b, v):
    s = slice(c * BLOC, (c + 1) * BLOC)
    enc_l = enc[s]                                   # [BLOC, L, H]
    return {
        "encT": np.ascontiguousarray(enc_l.transpose(0, 2, 1)),
        "attn_wT": np.ascontiguousarray(attn_w.T),
        "w_effT": np.ascontiguousarray(w_eff.T),
        "dec_wT": np.ascontiguousarray(dec_w.T),
        "vT": np.ascontiguousarray(v[s].T),
        "covT": np.ascontiguousarray(coverage[s].T),
        "hidT": np.ascontiguousarray(hidden[s].T),
        "cov_in": np.ascontiguousarray(coverage[s]),
        "maskb": np.ascontiguousarray(maskf[s]),
        "attn_b_r": np.ascontiguousarray(attn_b[None, :]),
        "dec_b_r": np.ascontiguousarray(dec_b[None, :]),
        "cvg_b_r": np.ascontiguousarray(cvg_b[None, :]),
    }


def kernel(encoder_outputs, attn_mask, hidden, coverage,
           attn_w, attn_b, dec_w, dec_b, cvg_w, cvg_b, v):
    enc = np.asarray(encoder_outputs, dtype=np.float32)
    mask = np.asarray(attn_mask)
    hidden = np.asarray(hidden, dtype=np.float32)
    coverage = np.asarray(coverage, dtype=np.float32)
    attn_w = np.asarray(attn_w, dtype=np.float32)
    attn_b = np.asarray(attn_b, dtype=np.float32)
    dec_w = np.asarray(dec_w, dtype=np.float32)
    dec_b = np.asarray(dec_b, dtype=np.float32)
    cvg_b = np.asarray(cvg_b, dtype=np.float32)
    v = np.asarray(v, dtype=np.float32)
    # 'same' padding with kernel (1, H) on a single pixel: only the center
    # column of the conv weight is ever active.
    center = (H - 1) // 2
    w_eff = np.asarray(cvg_w[:, :, 0, center], dtype=np.float32)
    maskf = np.where(mask == 1, np.float32(0.0), np.float32(-1e38))

    nc = _get_program()
    in_maps = [
        _prep_core_inputs(c, enc, maskf, hidden, coverage, attn_w, attn_b,
                          dec_w, dec_b, w_eff, cvg_b, v)
        for c in range(NCORES)
    ]
    trace = os.environ.get("KERNEL_TRACE", "") == "1"
    res = run_bass_kernel_spmd(nc, in_maps, core_ids=list(range(NCORES)),
                               trace=trace)
    if trace and res.exec_time_ns is not None:
        _CACHE["exec_time_ns"] = res.exec_time_ns
        _CACHE["mean_exec_time_ns"] = res.mean_exec_time_ns
        _CACHE["trace"] = res.instructions_and_trace

    ctx = np.empty((B, H), np.float32)
    aw = np.empty((B, L), np.float32)
    ncov = np.empty((B, L), np.float32)
    for c in range(NCORES):
        r = res.results[c]
        s = slice(c * BLOC, (c + 1) * BLOC)
        aw[s] = r["aw_out"]
        ncov[s] = r["ncov_out"]
        # ctxT_out[p, k, b] -> ctx[b, k*128+p]
        ctx[s] = r["ctxT_out"].transpose(2, 1, 0).reshape(BLOC, H)
    return ctx, aw, ncov


# revision 31
# speedup vs baseline: 1.3753x; 1.3753x over previous
"""Trainium2 Bass kernel for nn_AttnCalc (coverage attention).

Contract: kernel(**inputs) takes FULL unsharded numpy inputs, distributes
batch-parallel across 8 NeuronCores, returns the full
(context_vector, attn_weights, new_coverage) tuple like the reference.

Math per batch b:
  enc_feat = enc[b] @ attn_w.T + attn_b          [L,H]
  dec_feat = dec_w @ hidden[b] + dec_b           [H]
  cov_feat = w_eff @ coverage[b] + cvg_b         [L]   (w_eff = cvg_w[:,:,0,(H-1)//2])
  feats    = tanh(enc_feat + dec_feat + cov_feat[:,None])
  scores   = feats @ v[b]  (masked, softmax over L) -> aw
  new_cov  = coverage[b] + aw
  context  = aw @ enc[b]                         [H]

Device layout (per core, BLOC=8 batches):
  [*,L]-major work uses tiles [128 (H-chunk), L] (transposed layout,
  contraction over H on PE partitions).  cov_feat and dec_feat(+biases)
  are folded into the enc_feat PSUM accumulation (K=1 rank-1 matmuls)
  and the tanh activation bias.  The context contraction (over L) runs
  on the Vector engine as tensor_tensor_reduce against a DMA-broadcast
  of aw, so encoder_outputs is only loaded once, in transposed form.

The target walrus build allows only ONE semaphore wait per TPB compute
instruction, so the program is arranged so every compute op has at most
one cross-engine dependency; tiny "absorber" ops (1x1 matmul / copy)
pick up the remaining waits ahead of real work.
"""

import sys
import os

sys.path.insert(0, "/opt/trn_rl_repo")

import numpy as np

import concourse.bass as bass
import concourse.tile as tile
from concourse import mybir
from concourse.bass_utils import run_bass_kernel_spmd
from concourse.tile_rust import add_dep_helper

B, L, H = 64, 512, 512
NCORES = 8
BLOC = B // NCORES          # batches per core
P = 128                     # SBUF partitions
PC = H // P                 # 128-chunks along H (== along L)
F32 = mybir.dt.float32
F32R = mybir.dt.float32r
Tanh = mybir.ActivationFunctionType.Tanh
Exp = mybir.ActivationFunctionType.Exp
Copy = mybir.ActivationFunctionType.Copy

_CACHE = {}


def _build_program():
    nc = bass.Bass()

    encT = nc.declare_dram_parameter("encT", [BLOC, H, L], F32R, isOutput=False)
    attn_wT = nc.declare_dram_parameter("attn_wT", [H, H], F32R, isOutput=False)
    w_effT = nc.declare_dram_parameter("w_effT", [L, L], F32, isOutput=False)
    dec_wT = nc.declare_dram_parameter("dec_wT", [H, H], F32, isOutput=False)
    vT = nc.declare_dram_parameter("vT", [H, BLOC], F32R, isOutput=False)
    covT = nc.declare_dram_parameter("covT", [L, BLOC], F32, isOutput=False)
    hidT = nc.declare_dram_parameter("hidT", [H, BLOC], F32, isOutput=False)
    cov_in = nc.declare_dram_parameter("cov_in", [BLOC, L], F32, isOutput=False)
    maskb = nc.declare_dram_parameter("maskb", [BLOC, L], F32, isOutput=False)
    attn_b_r = nc.declare_dram_parameter("attn_b_r", [1, H], F32, isOutput=False)
    dec_b_r = nc.declare_dram_parameter("dec_b_r", [1, H], F32, isOutput=False)
    cvg_b_r = nc.declare_dram_parameter("cvg_b_r", [1, L], F32, isOutput=False)

    aw_out = nc.declare_dram_parameter("aw_out", [BLOC, L], F32, isOutput=True)
    ncov_out = nc.declare_dram_parameter("ncov_out", [BLOC, L], F32, isOutput=True)
    ctxT_out = nc.declare_dram_parameter("ctxT_out", [P, PC, BLOC], F32, isOutput=True)

    def row3(dram2d):
        # [BLOC, L] dram -> [1, BLOC, L] AP so rows can live on partition 0
        return dram2d[:, :].rearrange("b l -> (b l)")[None].rearrange(
            "o (b l) -> o b l", b=BLOC)

    with tile.TileContext(nc) as tc:
        with (
            tc.tile_pool(name="const", bufs=1) as const,
            tc.tile_pool(name="enc", bufs=2) as epool,
            tc.tile_pool(name="feat", bufs=2) as fpool,
            tc.tile_pool(name="awb", bufs=2) as bpool,
            tc.tile_pool(name="prod", bufs=2) as prpool,
            tc.tile_pool(name="eps", bufs=4, space=bass.MemorySpace.PSUM) as ppool,
            tc.tile_pool(name="scps", bufs=2, space=bass.MemorySpace.PSUM) as scpool,
            tc.tile_pool(name="setps", bufs=1, space=bass.MemorySpace.PSUM) as stpool,
            tc.tile_pool(name="dumps", bufs=1, space=bass.MemorySpace.PSUM) as dumpool,
            tc.tile_pool(name="dram", bufs=1, space=bass.MemorySpace.DRAM) as dpool,
        ):
            # -------- wait absorbers (1x1 ops that pick up semaphore waits
            # so real compute ops never need more than one) --------
            dum_t = dumpool.tile([1, 64], F32, tag="dummy")
            dve_dum = const.tile([1, 256], F32)
            act_dum = const.tile([1, 256], F32)
            _ctr = {"pe": 0, "dve": 0, "act": 0}

            def pe_abs(ap):
                i = _ctr["pe"] = (_ctr["pe"] + 1) % 64
                ap = ap.bitcast(F32)
                return nc.tensor.matmul(dum_t[0:1, i:i + 1], ap, ap,
                                        start=True, stop=True)

            def dve_abs(ap):
                i = _ctr["dve"] = (_ctr["dve"] + 1) % 256
                return nc.vector.tensor_copy(dve_dum[0:1, i:i + 1], ap)

            def act_abs(ap):
                i = _ctr["act"] = (_ctr["act"] + 1) % 256
                return nc.scalar.activation(act_dum[0:1, i:i + 1], ap, Copy)

            def pin(real, *deps):
                for d in deps:
                    add_dep_helper(real.ins, d.ins, sync=False,
                                   reason="absorber ordering")

            # ---------------- constants ----------------
            wA = const.tile([P, PC, H], F32R)  # attn_wT  [h=k*128+p][o]
            wC = const.tile([P, PC, L], F32)   # w_effT   [l'=k*128+p][l]
            wD = const.tile([P, PC, H], F32)   # dec_wT   [h=k*128+p][o]
            vS = const.tile([P, PC, BLOC], F32R)
            cS = const.tile([P, PC, BLOC], F32)
            hS = const.tile([P, PC, BLOC], F32)
            nc.sync.dma_start(out=wA, in_=attn_wT[:, :].rearrange("(k p) o -> p k o", p=P))
            nc.sync.dma_start(out=wC, in_=w_effT[:, :].rearrange("(k p) o -> p k o", p=P))
            nc.sync.dma_start(out=wD, in_=dec_wT[:, :].rearrange("(k p) o -> p k o", p=P))
            nc.sync.dma_start(out=vS, in_=vT[:, :].rearrange("(k p) b -> p k b", p=P))
            nc.sync.dma_start(out=cS, in_=covT[:, :].rearrange("(k p) b -> p k b", p=P))
            nc.sync.dma_start(out=hS, in_=hidT[:, :].rearrange("(k p) b -> p k b", p=P))

            ab_row = const.tile([1, H], F32)
            db_row = const.tile([1, H], F32)
            cb_row = const.tile([1, L], F32)
            nc.sync.dma_start(out=ab_row, in_=attn_b_r[:, :])
            nc.sync.dma_start(out=db_row, in_=dec_b_r[:, :])
            nc.sync.dma_start(out=cb_row, in_=cvg_b_r[:, :])

            ones_b = const.tile([1, BLOC], F32)
            nc.vector.memset(ones_b, 1.0)
            ones_p = const.tile([1, P], F32)
            nc.vector.memset(ones_p, 1.0)
            d_ab = dve_abs(ab_row[0:1, 0:1])
            d_db = dve_abs(db_row[0:1, 0:1])
            brow = const.tile([1, H], F32)     # attn_b + dec_b
            badd = nc.vector.tensor_add(brow, ab_row, db_row)
            pin(badd, d_ab, d_db)

            # per-batch rows as disjoint slices of persistent [1,BLOC,L]
            # tiles on partition 0 (no slot reuse -> no release waits)
            mb = const.tile([1, BLOC, L], F32)
            covin = const.tile([1, BLOC, L], F32)
            nc.sync.dma_start(out=mb, in_=row3(maskb))
            nc.sync.dma_start(out=covin, in_=row3(cov_in))
            d_mb = dve_abs(mb[0:1, 0, 0:1])
            d_cvn = dve_abs(covin[0:1, 0, 0:1])

            sc = const.tile([1, BLOC, L], F32)      # scores -> exp -> aw
            nmx = const.tile([1, BLOC, 1], F32)
            se = const.tile([1, BLOC, 1], F32)
            rse = const.tile([1, BLOC, 1], F32)
            cov_sb = const.tile([1, BLOC, L], F32)  # cov_feat rows
            ctx_all = const.tile([P, PC, BLOC], F32)

            # ---------------- setup: cov_feat rows ----------------
            d_wC = pe_abs(wC[0:1, 0, 0:1])
            d_cS = pe_abs(cS[0:1, 0, 0:1])
            d_cb = pe_abs(cb_row[0:1, 0:1])
            d_on = pe_abs(ones_b[0:1, 0:1])
            cov_ps = stpool.tile([BLOC, L], F32, tag="setup")
            for k in range(PC):
                mm = nc.tensor.matmul(cov_ps, cS[:, k, :], wC[:, k, :],
                                      start=(k == 0), stop=False)
                if k == 0:
                    pin(mm, d_wC, d_cS)
            mm = nc.tensor.matmul(cov_ps, ones_b[:, :], cb_row[:, :],
                                  start=False, stop=True)
            pin(mm, d_cb, d_on)
            # bounce through DRAM to turn [BLOC(part), L] into row-major
            # [1, BLOC, L] on partition 0
            cov_tmp = const.tile([BLOC, L], F32)
            nc.vector.tensor_copy(cov_tmp, cov_ps)
            cov_dram = dpool.tile([BLOC, L], F32)
            nc.sync.dma_start(out=cov_dram[:, :], in_=cov_tmp)
            nc.sync.dma_start(out=cov_sb, in_=row3(cov_dram))
            d_cov = pe_abs(cov_sb[0:1, 0, 0:1])

            # ---------------- setup: tanh bias rows ----------------
            # bias_sb[p, o_chunk, b] = (dec_w @ hidden[b] + dec_b + attn_b)[o]
            d_wD = pe_abs(wD[0:1, 0, 0:1])
            d_hS = pe_abs(hS[0:1, 0, 0:1])
            d_br = pe_abs(brow[0:1, 0:1])
            dps = stpool.tile([P, PC, BLOC], F32, tag="setup")
            for o in range(PC):
                for k in range(PC):
                    mm = nc.tensor.matmul(dps[:, o, :], wD[:, k, o * P:(o + 1) * P],
                                          hS[:, k, :], start=(k == 0), stop=False)
                    if k == 0:
                        pin(mm, d_wD, d_hS)
                bmm = nc.tensor.matmul(dps[:, o, :], brow[0:1, o * P:(o + 1) * P],
                                       ones_b[:, :], start=False, stop=True)
                pin(bmm, d_br)
            bias_sb = const.tile([P, PC, BLOC], F32)
            nc.scalar.copy(bias_sb, dps)
            a_bias = act_abs(bias_sb[0:1, 0, 0:1])
            d_vS = pe_abs(vS[0:1, 0, 0:1])
            d_wA = pe_abs(wA[0:1, 0, 0:1])
            d_op = pe_abs(ones_p[0:1, 0:1])

            # ---------------- main per-batch pipeline ----------------
            prev_exp = None
            prev_df = None
            for b in range(BLOC):
                # wait-free SP-stream slots for the legalizer, kept near
                # this batch's DMAs by nosync pins
                sps = [nc.sync.nop(nofuse=True) for _ in range(3)]
                if prev_df is not None:
                    pin(sps[0], prev_df)
                pin(sps[1], sps[0])
                pin(sps[2], sps[1])
                eT = epool.tile([P, PC, L], F32R)
                eT_dma = nc.sync.dma_start(
                    out=eT, in_=encT[b].rearrange("(k p) l -> p k l", p=P))
                pin(eT_dma, sps[2])
                d_e = pe_abs(eT[0:1, 0, 0:1])       # eT DMA wait -> PE
                v_e = dve_abs(eT[0:1, 0, 0:1])      # eT DMA wait -> DVE

                # cov_feat row broadcast across partitions for the DVE add
                gp_cv = [nc.gpsimd.nop(nofuse=True) for _ in range(2)]
                pin(gp_cv[0], d_e)
                pin(gp_cv[1], gp_cv[0])
                cb_t = bpool.tile([P, L], F32, tag="covb")
                cv_dma = nc.gpsimd.dma_start(
                    out=cb_t, in_=cov_dram[b:b + 1, :].to_broadcast([P, L]))
                pin(cv_dma, gp_cv[1])
                v_cb = dve_abs(cb_t[0:1, 0:1])

                # enc_featT[o,l] in PSUM; +cov on DVE; tanh(+bias) -> ft
                # a_slot is a wait-free ACT op the legalizer can move
                # bounced same-engine waits onto; anchor it after the
                # previous batch's exp so it lands between batches
                a_slot = act_abs(ones_b[0:1, 0:1])
                a_slot2 = act_abs(ones_b[0:1, 0:1])
                if prev_exp is not None:
                    pin(a_slot, prev_exp)
                pin(a_slot2, a_slot)
                ft = fpool.tile([P, PC, L], F32R)
                first_th = None
                for o in range(PC):
                    ps = ppool.tile([P, L], F32, tag="encps")
                    for k in range(PC):
                        mm = nc.tensor.matmul(ps, wA[:, k, o * P:(o + 1) * P],
                                              eT[:, k, :], start=(k == 0),
                                              stop=(k == 3))
                        if k == 0:
                            pin(mm, d_e)
                            if b == 0:
                                pin(mm, d_wA)
                    ta = nc.vector.tensor_add(ft[:, o, :], ps, cb_t)
                    if o == 0:
                        pin(ta, v_cb)
                    th = nc.scalar.activation(
                        out=ft[:, o, :], in_=ft[:, o, :], func=Tanh,
                        bias=bias_sb[:, o, b:b + 1], scale=1.0)
                    if first_th is None:
                        first_th = th
                        pin(th, a_slot2)
                    if b == 0 and o == 0:
                        pin(th, a_bias)

                # scores[l] = sum_o feats[o,l] * v[b,o]
                d_f = pe_abs(ft[0:1, 0, 0:1])
                prev_df = d_f
                sc_ps = scpool.tile([1, L], F32)
                for k in range(PC):
                    mm = nc.tensor.matmul(sc_ps, vS[:, k, b:b + 1],
                                          ft[:, k, :],
                                          start=(k == 0), stop=(k == 3))
                    if k == 0:
                        pin(mm, d_f)
                        if b == 0:
                            pin(mm, d_vS)

                # masked softmax over l, in sc row b (aw ends up there)
                scr = sc[0:1, b, :]
                madd = nc.vector.tensor_add(scr, sc_ps, mb[0:1, b, :])
                if b == 0:
                    pin(madd, d_mb)
                nc.vector.tensor_reduce(out=nmx[0:1, b, :], in_=scr,
                                        axis=mybir.AxisListType.X,
                                        op=mybir.AluOpType.max, negate=True)
                prev_exp = nc.scalar.activation(out=scr, in_=scr, func=Exp,
                                                bias=nmx[0:1, b, :], scale=1.0,
                                                accum_out=se[0:1, b, :])
                nc.vector.reciprocal(rse[0:1, b, :], se[0:1, b, :])
                nc.vector.tensor_scalar_mul(scr, scr, rse[0:1, b, :])

                # aw row out + broadcast back for the context reduction
                sp_aw = nc.sync.nop(nofuse=True)
                pin(sp_aw, mm)           # anchor inside this batch
                aw_dma = nc.sync.dma_start(out=aw_out[b:b + 1, :], in_=scr)
                pin(aw_dma, sp_aw)
                gp_slots = [nc.gpsimd.nop(nofuse=True) for _ in range(2)]
                pin(gp_slots[0], mm)
                pin(gp_slots[1], gp_slots[0])
                aw_b = bpool.tile([P, L], F32, tag="awb")
                bc_dma = nc.gpsimd.dma_start(
                    out=aw_b, in_=aw_out[b:b + 1, :].to_broadcast([P, L]))
                pin(bc_dma, gp_slots[1])

                # new_coverage row (in place over covin row)
                ncadd = nc.vector.tensor_add(covin[0:1, b, :],
                                              covin[0:1, b, :], scr)
                if b == 0:
                    pin(ncadd, d_cvn)
                sp_nc = nc.sync.nop(nofuse=True)
                pin(sp_nc, aw_dma)
                nc_dma = nc.sync.dma_start(out=ncov_out[b:b + 1, :],
                                           in_=covin[0:1, b, :])
                pin(nc_dma, sp_nc)

                # context[h] = sum_l aw[l] * encT[h,l] on DVE
                # (tensor_tensor_reduce is a raw-ISA op this walrus cannot
                # encode, so use mul + reduce_sum)
                for o in range(PC):
                    prod = prpool.tile([P, L], F32, tag="prod")
                    tm = nc.vector.tensor_mul(prod, eT[:, o, :].bitcast(F32),
                                               aw_b)
                    if o == 0:
                        pin(tm, v_e)
                    nc.vector.reduce_sum(out=ctx_all[:, o, b:b + 1], in_=prod,
                                         axis=mybir.AxisListType.X)

            ctx_dma = nc.sync.dma_start(out=ctxT_out[:, :, :], in_=ctx_all)

            # tail landing slots: the framework kernel-tail drain waits on
            # every engine/queue semaphore at once; give the legalizer SP
            # instructions to spread those waits over
            tail = ctx_dma
            for _ in range(22):
                n = nc.sync.nop(nofuse=True)
                pin(n, tail)
                tail = n

    _legalize_waits(nc)
    return nc


# The nix walrus build (setupSyncWait) accepts only ONE sync wait per TPB
# instruction (compute and DMA alike).  Tile can emit several.  Because the
# committed instruction order is a topological order of the dependency
# graph, a wait whose producing semaphore update completes at block index p
# can be safely carried by ANY same-engine instruction at index > p that
# precedes the original carrier: engines execute in order, so the original
# instruction still starts after the wait is satisfied, and the producer
# (committed before the new carrier) cannot depend on it -- no deadlock.
# Assign waits to instructions as an interval matching problem.
def _legalize_waits(nc):
    import concourse.mybir as _mb

    fn = nc.m.functions[0]
    stuck = []
    NO_LANDING = ("InstISA", "InstEventSemaphore", "InstUnconditionalBranch",
                  "InstCall", "InstRegisterMove", "InstHalt")
    # one global stream per engine, blocks concatenated in order (engines
    # branch from block to block in order, so per-engine execution order is
    # block order)
    insts = []
    for blk in fn.blocks:
        insts.extend(blk.instructions)

    sem_hist = {}
    cum = {}
    streams = {}
    for i, inst in enumerate(insts):
        si = inst.sync_info
        if si is not None:
            for u in si.on_update:
                cum[u.id] = cum.get(u.id, 0) + u.update_value
                sem_hist.setdefault(u.id, []).append((i, cum[u.id]))
        streams.setdefault(inst.engine, []).append(i)

    def producer_idx(w):
        hist = sem_hist.get(w.id)
        if hist is None:
            return None            # unknown semaphore: not movable
        for i, v in hist:
            if v >= w.wait_value:
                return i
        return None

    for eng, stream in streams.items():
        movable_spos = []
        pinned = {}                # spos -> unmovable waits
        waits = []                 # (carrier_spos, producer_bidx, wait)
        has_multi = False
        for spos, i in enumerate(stream):
            inst = insts[i]
            si = inst.sync_info
            ws = list(si.on_wait) if si is not None else []
            if len(ws) > 1:
                has_multi = True

            def mov(w):
                if w.wait_reg is not None or w.wait_value <= 0:
                    return False
                p = producer_idx(w)
                return p is not None and p < i
            special = inst.__class__.__name__ in NO_LANDING
            unmov = [w for w in ws if special or not mov(w)]
            if unmov:
                pinned[spos] = unmov
            elif not special:
                movable_spos.append(spos)
            if special:
                continue
            best = {}
            for w in ws:
                if not mov(w):
                    continue
                if w.id not in best or w.wait_value > best[w.id].wait_value:
                    best[w.id] = w
            for w in best.values():
                waits.append((spos, producer_idx(w), w))
        if not has_multi:
            continue
        bidx_of = {spos: stream[spos] for spos in range(len(stream))}
        free = sorted(movable_spos)
        assign = {}
        for carrier, pbidx, w in sorted(waits, key=lambda t: (t[0], -t[1])):
            chosen = None
            for spos in reversed(free):
                if spos > carrier:
                    continue
                if bidx_of[spos] <= pbidx:
                    break
                chosen = spos
                break
            if chosen is None:
                stuck.append((insts[stream[carrier]].name,
                              insts[stream[carrier]].__class__.__name__,
                              w.ant_name, w.wait_value))
                continue
            free.remove(chosen)
            assign.setdefault(chosen, []).append(w)
        for spos in range(len(stream)):
            inst = insts[stream[spos]]
            si = inst.sync_info
            ups = list(si.on_update) if si is not None else []
            new_w = pinned.get(spos, []) + assign.get(spos, [])
            if si is None and not new_w:
                continue
            inst.sync_info = _mb.SyncInfo(on_wait=new_w, on_update=ups)
    if stuck:
        raise RuntimeError(f"wait legalization failed: {stuck[:8]}")


def _get_program():
    if "nc" not in _CACHE:
        _CACHE["nc"] = _build_program()
    return _CACHE["nc"]


def _prep_core_inputs(c, enc, maskf, hidden, coverage, attn_w, attn_b,
                      dec_w, dec_b, w_eff, cvg_b, v):
    s = slice(c * BLOC, (c + 1) * BLOC)
    enc_l = enc[s]                                   # [BLOC, L, H]
    return {
        "encT": np.ascontiguousarray(enc_l.transpose(0, 2, 1)),
        "attn_wT": np.ascontiguousarray(attn_w.T),
        "w_effT": np.ascontiguousarray(w_eff.T),
        "dec_wT": np.ascontiguousarray(dec_w.T),
        "vT": np.ascontiguousarray(v[s].T),
        "covT": np.ascontiguousarray(coverage[s].T),
        "hidT": np.ascontiguousarray(hidden[s].T),
        "cov_in": np.ascontiguousarray(coverage[s]),
        "maskb": np.ascontiguousarray(maskf[s]),
        "attn_b_r": np.ascontiguousarray(attn_b[None, :]),
        "dec_b_r": np.ascontiguousarray(dec_b[None, :]),
        "cvg_b_r": np.ascontiguousarray(cvg_b[None, :]),
    }


def kernel(encoder_outputs, attn_mask, hidden, coverage,
           attn_w, attn_b, dec_w, dec_b, cvg_w, cvg_b, v):
    enc = np.asarray(encoder_outputs, dtype=np.float32)
    mask = np.asarray(attn_mask)
    hidden = np.asarray(hidden, dtype=np.float32)
    coverage = np.asarray(coverage, dtype=np.float32)
    attn_w = np.asarray(attn_w, dtype=np.float32)
    attn_b = np.asarray(attn_b, dtype=np.float32)
    dec_w = np.asarray(dec_w, dtype=np.float32)
    dec_b = np.asarray(dec_b, dtype=np.float32)
    cvg_b = np.asarray(cvg_b, dtype=np.float32)
    v = np.asarray(v, dtype=np.float32)
    # 'same' padding with kernel (1, H) on a single pixel: only the center
    # column of the conv weight is ever active.
    center = (H - 1) // 2
    w_eff = np.asarray(cvg_w[:, :, 0, center], dtype=np.float32)
    maskf = np.where(mask == 1, np.float32(0.0), np.float32(-1e38))

    nc = _get_program()
    in_maps = [
        _prep_core_inputs(c, enc, maskf, hidden, coverage, attn_w, attn_b,
                          dec_w, dec_b, w_eff, cvg_b, v)
        for c in range(NCORES)
    ]
    trace = os.environ.get("KERNEL_TRACE", "") == "1"
    res = run_bass_kernel_spmd(nc, in_maps, core_ids=list(range(NCORES)),
                               trace=trace)
    if trace and res.exec_time_ns is not None:
        _CACHE["exec_time_ns"] = res.exec_time_ns
        _CACHE["mean_exec_time_ns"] = res.mean_exec_time_ns
        _CACHE["trace"] = res.instructions_and_trace

    ctx = np.empty((B, H), np.float32)
    aw = np.empty((B, L), np.float32)
    ncov = np.empty((B, L), np.float32)
    for c in range(NCORES):
        r = res.results[c]
        s = slice(c * BLOC, (c + 1) * BLOC)
        aw[s] = r["aw_out"]
        ncov[s] = r["ncov_out"]
        # ctxT_out[p, k, b] -> ctx[b, k*128+p]
        ctx[s] = r["ctxT_out"].transpose(2, 1, 0).reshape(BLOC, H)
    return ctx, aw, ncov


# revision 32
# speedup vs baseline: 1.4267x; 1.0374x over previous
"""Trainium2 Bass kernel for nn_AttnCalc (coverage attention).

Contract: kernel(**inputs) takes FULL unsharded numpy inputs, distributes
batch-parallel across 8 NeuronCores, returns the full
(context_vector, attn_weights, new_coverage) tuple like the reference.

Math per batch b:
  enc_feat = enc[b] @ attn_w.T + attn_b          [L,H]
  dec_feat = dec_w @ hidden[b] + dec_b           [H]
  cov_feat = w_eff @ coverage[b] + cvg_b         [L]   (w_eff = cvg_w[:,:,0,(H-1)//2])
  feats    = tanh(enc_feat + dec_feat + cov_feat[:,None])
  scores   = feats @ v[b]  (masked, softmax over L) -> aw
  new_cov  = coverage[b] + aw
  context  = aw @ enc[b]                         [H]

Device layout (per core, BLOC=8 batches):
  [*,L]-major work uses tiles [128 (H-chunk), L] (transposed layout,
  contraction over H on PE partitions).  cov_feat and dec_feat(+biases)
  are folded into the enc_feat PSUM accumulation (K=1 rank-1 matmuls)
  and the tanh activation bias.  The context contraction (over L) runs
  on the Vector engine as tensor_tensor_reduce against a DMA-broadcast
  of aw, so encoder_outputs is only loaded once, in transposed form.

The target walrus build allows only ONE semaphore wait per TPB compute
instruction, so the program is arranged so every compute op has at most
one cross-engine dependency; tiny "absorber" ops (1x1 matmul / copy)
pick up the remaining waits ahead of real work.
"""

import sys
import os

sys.path.insert(0, "/opt/trn_rl_repo")

import numpy as np

import concourse.bass as bass
import concourse.tile as tile
from concourse import mybir
from concourse.bass_utils import run_bass_kernel_spmd
from concourse.tile_rust import add_dep_helper

B, L, H = 64, 512, 512
NCORES = 8
BLOC = B // NCORES          # batches per core
P = 128                     # SBUF partitions
PC = H // P                 # 128-chunks along H (== along L)
F32 = mybir.dt.float32
F32R = mybir.dt.float32r
Tanh = mybir.ActivationFunctionType.Tanh
Exp = mybir.ActivationFunctionType.Exp
Copy = mybir.ActivationFunctionType.Copy

_CACHE = {}


def _build_program():
    nc = bass.Bass()

    encT = nc.declare_dram_parameter("encT", [BLOC, H, L], F32R, isOutput=False)
    attn_wT = nc.declare_dram_parameter("attn_wT", [H, H], F32R, isOutput=False)
    w_effT = nc.declare_dram_parameter("w_effT", [L, L], F32, isOutput=False)
    dec_wT = nc.declare_dram_parameter("dec_wT", [H, H], F32, isOutput=False)
    vT = nc.declare_dram_parameter("vT", [H, BLOC], F32R, isOutput=False)
    covT = nc.declare_dram_parameter("covT", [L, BLOC], F32, isOutput=False)
    hidT = nc.declare_dram_parameter("hidT", [H, BLOC], F32, isOutput=False)
    cov_in = nc.declare_dram_parameter("cov_in", [BLOC, L], F32, isOutput=False)
    maskb = nc.declare_dram_parameter("maskb", [BLOC, L], F32, isOutput=False)
    attn_b_r = nc.declare_dram_parameter("attn_b_r", [1, H], F32, isOutput=False)
    dec_b_r = nc.declare_dram_parameter("dec_b_r", [1, H], F32, isOutput=False)
    cvg_b_r = nc.declare_dram_parameter("cvg_b_r", [1, L], F32, isOutput=False)

    aw_out = nc.declare_dram_parameter("aw_out", [BLOC, L], F32, isOutput=True)
    ncov_out = nc.declare_dram_parameter("ncov_out", [BLOC, L], F32, isOutput=True)
    ctxT_out = nc.declare_dram_parameter("ctxT_out", [P, PC, BLOC], F32, isOutput=True)

    def row3(dram2d):
        # [BLOC, L] dram -> [1, BLOC, L] AP so rows can live on partition 0
        return dram2d[:, :].rearrange("b l -> (b l)")[None].rearrange(
            "o (b l) -> o b l", b=BLOC)

    with tile.TileContext(nc) as tc:
        with (
            tc.tile_pool(name="const", bufs=1) as const,
            tc.tile_pool(name="enc", bufs=3) as epool,
            tc.tile_pool(name="feat", bufs=3) as fpool,
            tc.tile_pool(name="awb", bufs=3) as bpool,
            tc.tile_pool(name="prod", bufs=2) as prpool,
            tc.tile_pool(name="eps", bufs=4, space=bass.MemorySpace.PSUM) as ppool,
            tc.tile_pool(name="scps", bufs=2, space=bass.MemorySpace.PSUM) as scpool,
            tc.tile_pool(name="setps", bufs=1, space=bass.MemorySpace.PSUM) as stpool,
            tc.tile_pool(name="dumps", bufs=1, space=bass.MemorySpace.PSUM) as dumpool,
            tc.tile_pool(name="dram", bufs=1, space=bass.MemorySpace.DRAM) as dpool,
        ):
            # -------- wait absorbers (1x1 ops that pick up semaphore waits
            # so real compute ops never need more than one) --------
            dum_t = dumpool.tile([1, 64], F32, tag="dummy")
            dve_dum = const.tile([1, 256], F32)
            act_dum = const.tile([1, 256], F32)
            _ctr = {"pe": 0, "dve": 0, "act": 0}

            def pe_abs(ap):
                i = _ctr["pe"] = (_ctr["pe"] + 1) % 64
                ap = ap.bitcast(F32)
                return nc.tensor.matmul(dum_t[0:1, i:i + 1], ap, ap,
                                        start=True, stop=True)

            def dve_abs(ap):
                i = _ctr["dve"] = (_ctr["dve"] + 1) % 256
                return nc.vector.tensor_copy(dve_dum[0:1, i:i + 1], ap)

            def act_abs(ap):
                i = _ctr["act"] = (_ctr["act"] + 1) % 256
                return nc.scalar.activation(act_dum[0:1, i:i + 1], ap, Copy)

            def pin(real, *deps):
                for d in deps:
                    add_dep_helper(real.ins, d.ins, sync=False,
                                   reason="absorber ordering")

            # ---------------- constants ----------------
            wA = const.tile([P, PC, H], F32R)  # attn_wT  [h=k*128+p][o]
            wC = const.tile([P, PC, L], F32)   # w_effT   [l'=k*128+p][l]
            wD = const.tile([P, PC, H], F32)   # dec_wT   [h=k*128+p][o]
            vS = const.tile([P, PC, BLOC], F32R)
            cS = const.tile([P, PC, BLOC], F32)
            hS = const.tile([P, PC, BLOC], F32)
            nc.sync.dma_start(out=wA, in_=attn_wT[:, :].rearrange("(k p) o -> p k o", p=P))
            nc.sync.dma_start(out=wC, in_=w_effT[:, :].rearrange("(k p) o -> p k o", p=P))
            nc.sync.dma_start(out=wD, in_=dec_wT[:, :].rearrange("(k p) o -> p k o", p=P))
            nc.sync.dma_start(out=vS, in_=vT[:, :].rearrange("(k p) b -> p k b", p=P))
            nc.sync.dma_start(out=cS, in_=covT[:, :].rearrange("(k p) b -> p k b", p=P))
            nc.sync.dma_start(out=hS, in_=hidT[:, :].rearrange("(k p) b -> p k b", p=P))

            ab_row = const.tile([1, H], F32)
            db_row = const.tile([1, H], F32)
            cb_row = const.tile([1, L], F32)
            nc.sync.dma_start(out=ab_row, in_=attn_b_r[:, :])
            nc.sync.dma_start(out=db_row, in_=dec_b_r[:, :])
            nc.sync.dma_start(out=cb_row, in_=cvg_b_r[:, :])

            ones_b = const.tile([1, BLOC], F32)
            nc.vector.memset(ones_b, 1.0)
            ones_p = const.tile([1, P], F32)
            nc.vector.memset(ones_p, 1.0)
            d_ab = dve_abs(ab_row[0:1, 0:1])
            d_db = dve_abs(db_row[0:1, 0:1])
            brow = const.tile([1, H], F32)     # attn_b + dec_b
            badd = nc.vector.tensor_add(brow, ab_row, db_row)
            pin(badd, d_ab, d_db)

            # per-batch rows as disjoint slices of persistent [1,BLOC,L]
            # tiles on partition 0 (no slot reuse -> no release waits)
            mb = const.tile([1, BLOC, L], F32)
            covin = const.tile([1, BLOC, L], F32)
            nc.sync.dma_start(out=mb, in_=row3(maskb))
            nc.sync.dma_start(out=covin, in_=row3(cov_in))
            d_mb = dve_abs(mb[0:1, 0, 0:1])
            d_cvn = dve_abs(covin[0:1, 0, 0:1])

            sc = const.tile([1, BLOC, L], F32)      # scores -> exp -> aw
            nmx = const.tile([1, BLOC, 1], F32)
            se = const.tile([1, BLOC, 1], F32)
            rse = const.tile([1, BLOC, 1], F32)
            cov_sb = const.tile([1, BLOC, L], F32)  # cov_feat rows
            ctx_all = const.tile([P, PC, BLOC], F32)

            # ---------------- setup: cov_feat rows ----------------
            d_wC = pe_abs(wC[0:1, 0, 0:1])
            d_cS = pe_abs(cS[0:1, 0, 0:1])
            d_cb = pe_abs(cb_row[0:1, 0:1])
            d_on = pe_abs(ones_b[0:1, 0:1])
            cov_ps = stpool.tile([BLOC, L], F32, tag="setup")
            for k in range(PC):
                mm = nc.tensor.matmul(cov_ps, cS[:, k, :], wC[:, k, :],
                                      start=(k == 0), stop=False)
                if k == 0:
                    pin(mm, d_wC, d_cS)
            mm = nc.tensor.matmul(cov_ps, ones_b[:, :], cb_row[:, :],
                                  start=False, stop=True)
            pin(mm, d_cb, d_on)
            # bounce through DRAM to turn [BLOC(part), L] into row-major
            # [1, BLOC, L] on partition 0
            cov_tmp = const.tile([BLOC, L], F32)
            nc.vector.tensor_copy(cov_tmp, cov_ps)
            cov_dram = dpool.tile([BLOC, L], F32)
            nc.sync.dma_start(out=cov_dram[:, :], in_=cov_tmp)
            nc.sync.dma_start(out=cov_sb, in_=row3(cov_dram))
            d_cov = pe_abs(cov_sb[0:1, 0, 0:1])

            # ---------------- setup: tanh bias rows ----------------
            # bias_sb[p, o_chunk, b] = (dec_w @ hidden[b] + dec_b + attn_b)[o]
            d_wD = pe_abs(wD[0:1, 0, 0:1])
            d_hS = pe_abs(hS[0:1, 0, 0:1])
            d_br = pe_abs(brow[0:1, 0:1])
            dps = stpool.tile([P, PC, BLOC], F32, tag="setup")
            for o in range(PC):
                for k in range(PC):
                    mm = nc.tensor.matmul(dps[:, o, :], wD[:, k, o * P:(o + 1) * P],
                                          hS[:, k, :], start=(k == 0), stop=False)
                    if k == 0:
                        pin(mm, d_wD, d_hS)
                bmm = nc.tensor.matmul(dps[:, o, :], brow[0:1, o * P:(o + 1) * P],
                                       ones_b[:, :], start=False, stop=True)
                pin(bmm, d_br)
            bias_sb = const.tile([P, PC, BLOC], F32)
            nc.scalar.copy(bias_sb, dps)
            a_bias = act_abs(bias_sb[0:1, 0, 0:1])
            d_vS = pe_abs(vS[0:1, 0, 0:1])
            d_wA = pe_abs(wA[0:1, 0, 0:1])
            d_op = pe_abs(ones_p[0:1, 0:1])

            # ---------------- main per-batch pipeline ----------------
            prev_exp = None
            prev_df = None
            for b in range(BLOC):
                # wait-free SP-stream slots for the legalizer, kept near
                # this batch's DMAs by nosync pins
                sps = [nc.sync.nop(nofuse=True) for _ in range(3)]
                if prev_df is not None:
                    pin(sps[0], prev_df)
                pin(sps[1], sps[0])
                pin(sps[2], sps[1])
                eT = epool.tile([P, PC, L], F32R)
                eT_dma = nc.sync.dma_start(
                    out=eT, in_=encT[b].rearrange("(k p) l -> p k l", p=P))
                pin(eT_dma, sps[2])
                d_e = pe_abs(eT[0:1, 0, 0:1])       # eT DMA wait -> PE
                v_e = dve_abs(eT[0:1, 0, 0:1])      # eT DMA wait -> DVE

                # cov_feat row broadcast across partitions for the DVE add
                gp_cv = [nc.gpsimd.nop(nofuse=True) for _ in range(2)]
                pin(gp_cv[0], d_e)
                pin(gp_cv[1], gp_cv[0])
                cb_t = bpool.tile([P, L], F32, tag="covb")
                cv_dma = nc.gpsimd.dma_start(
                    out=cb_t, in_=cov_dram[b:b + 1, :].to_broadcast([P, L]))
                pin(cv_dma, gp_cv[1])
                v_cb = dve_abs(cb_t[0:1, 0:1])

                # enc_featT[o,l] in PSUM; +cov on DVE; tanh(+bias) -> ft
                # a_slot is a wait-free ACT op the legalizer can move
                # bounced same-engine waits onto; anchor it after the
                # previous batch's exp so it lands between batches
                a_slot = act_abs(ones_b[0:1, 0:1])
                a_slot2 = act_abs(ones_b[0:1, 0:1])
                if prev_exp is not None:
                    pin(a_slot, prev_exp)
                pin(a_slot2, a_slot)
                ft = fpool.tile([P, PC, L], F32R)
                first_th = None
                for o in range(PC):
                    ps = ppool.tile([P, L], F32, tag="encps")
                    for k in range(PC):
                        mm = nc.tensor.matmul(ps, wA[:, k, o * P:(o + 1) * P],
                                              eT[:, k, :], start=(k == 0),
                                              stop=(k == 3))
                        if k == 0:
                            pin(mm, d_e)
                            if b == 0:
                                pin(mm, d_wA)
                    ta = nc.vector.tensor_add(ft[:, o, :], ps, cb_t)
                    if o == 0:
                        pin(ta, v_cb)
                    th = nc.scalar.activation(
                        out=ft[:, o, :], in_=ft[:, o, :], func=Tanh,
                        bias=bias_sb[:, o, b:b + 1], scale=1.0)
                    if first_th is None:
                        first_th = th
                        pin(th, a_slot2)
                    if b == 0 and o == 0:
                        pin(th, a_bias)

                # scores[l] = sum_o feats[o,l] * v[b,o]
                d_f = pe_abs(ft[0:1, 0, 0:1])
                prev_df = d_f
                sc_ps = scpool.tile([1, L], F32)
                for k in range(PC):
                    mm = nc.tensor.matmul(sc_ps, vS[:, k, b:b + 1],
                                          ft[:, k, :],
                                          start=(k == 0), stop=(k == 3))
                    if k == 0:
                        pin(mm, d_f)
                        if b == 0:
                            pin(mm, d_vS)

                # masked softmax over l, in sc row b (aw ends up there)
                scr = sc[0:1, b, :]
                madd = nc.vector.tensor_add(scr, sc_ps, mb[0:1, b, :])
                if b == 0:
                    pin(madd, d_mb)
                nc.vector.tensor_reduce(out=nmx[0:1, b, :], in_=scr,
                                        axis=mybir.AxisListType.X,
                                        op=mybir.AluOpType.max, negate=True)
                prev_exp = nc.scalar.activation(out=scr, in_=scr, func=Exp,
                                                bias=nmx[0:1, b, :], scale=1.0,
                                                accum_out=se[0:1, b, :])
                nc.vector.reciprocal(rse[0:1, b, :], se[0:1, b, :])
                nc.vector.tensor_scalar_mul(scr, scr, rse[0:1, b, :])

                # aw row out + broadcast back for the context reduction
                sp_aw = nc.sync.nop(nofuse=True)
                pin(sp_aw, mm)           # anchor inside this batch
                aw_dma = nc.sync.dma_start(out=aw_out[b:b + 1, :], in_=scr)
                pin(aw_dma, sp_aw)
                gp_slots = [nc.gpsimd.nop(nofuse=True) for _ in range(2)]
                pin(gp_slots[0], mm)
                pin(gp_slots[1], gp_slots[0])
                aw_b = bpool.tile([P, L], F32, tag="awb")
                bc_dma = nc.gpsimd.dma_start(
                    out=aw_b, in_=aw_out[b:b + 1, :].to_broadcast([P, L]))
                pin(bc_dma, gp_slots[1])

                # new_coverage row (in place over covin row)
                ncadd = nc.vector.tensor_add(covin[0:1, b, :],
                                              covin[0:1, b, :], scr)
                if b == 0:
                    pin(ncadd, d_cvn)
                sp_nc = nc.sync.nop(nofuse=True)
                pin(sp_nc, aw_dma)
                nc_dma = nc.sync.dma_start(out=ncov_out[b:b + 1, :],
                                           in_=covin[0:1, b, :])
                pin(nc_dma, sp_nc)

                # context[h] = sum_l aw[l] * encT[h,l] on DVE
                # (tensor_tensor_reduce is a raw-ISA op this walrus cannot
                # encode, so use mul + reduce_sum)
                for o in range(PC):
                    prod = prpool.tile([P, L], F32, tag="prod")
                    tm = nc.vector.tensor_mul(prod, eT[:, o, :].bitcast(F32),
                                               aw_b)
                    if o == 0:
                        pin(tm, v_e)
                    nc.vector.reduce_sum(out=ctx_all[:, o, b:b + 1], in_=prod,
                                         axis=mybir.AxisListType.X)

            ctx_dma = nc.sync.dma_start(out=ctxT_out[:, :, :], in_=ctx_all)

            # tail landing slots: the framework kernel-tail drain waits on
            # every engine/queue semaphore at once; give the legalizer SP
            # instructions to spread those waits over
            tail = ctx_dma
            for _ in range(22):
                n = nc.sync.nop(nofuse=True)
                pin(n, tail)
                tail = n

    _legalize_waits(nc)
    return nc


# The nix walrus build (setupSyncWait) accepts only ONE sync wait per TPB
# instruction (compute and DMA alike).  Tile can emit several.  Because the
# committed instruction order is a topological order of the dependency
# graph, a wait whose producing semaphore update completes at block index p
# can be safely carried by ANY same-engine instruction at index > p that
# precedes the original carrier: engines execute in order, so the original
# instruction still starts after the wait is satisfied, and the producer
# (committed before the new carrier) cannot depend on it -- no deadlock.
# Assign waits to instructions as an interval matching problem.
def _legalize_waits(nc):
    import concourse.mybir as _mb

    fn = nc.m.functions[0]
    stuck = []
    NO_LANDING = ("InstISA", "InstEventSemaphore", "InstUnconditionalBranch",
                  "InstCall", "InstRegisterMove", "InstHalt")
    # one global stream per engine, blocks concatenated in order (engines
    # branch from block to block in order, so per-engine execution order is
    # block order)
    insts = []
    for blk in fn.blocks:
        insts.extend(blk.instructions)

    sem_hist = {}
    cum = {}
    streams = {}
    for i, inst in enumerate(insts):
        si = inst.sync_info
        if si is not None:
            for u in si.on_update:
                cum[u.id] = cum.get(u.id, 0) + u.update_value
                sem_hist.setdefault(u.id, []).append((i, cum[u.id]))
        streams.setdefault(inst.engine, []).append(i)

    def producer_idx(w):
        hist = sem_hist.get(w.id)
        if hist is None:
            return None            # unknown semaphore: not movable
        for i, v in hist:
            if v >= w.wait_value:
                return i
        return None

    for eng, stream in streams.items():
        movable_spos = []
        pinned = {}                # spos -> unmovable waits
        waits = []                 # (carrier_spos, producer_bidx, wait)
        has_multi = False
        for spos, i in enumerate(stream):
            inst = insts[i]
            si = inst.sync_info
            ws = list(si.on_wait) if si is not None else []
            if len(ws) > 1:
                has_multi = True

            def mov(w):
                if w.wait_reg is not None or w.wait_value <= 0:
                    return False
                p = producer_idx(w)
                return p is not None and p < i
            special = inst.__class__.__name__ in NO_LANDING
            unmov = [w for w in ws if special or not mov(w)]
            if unmov:
                pinned[spos] = unmov
            elif not special:
                movable_spos.append(spos)
            if special:
                continue
            best = {}
            for w in ws:
                if not mov(w):
                    continue
                if w.id not in best or w.wait_value > best[w.id].wait_value:
                    best[w.id] = w
            for w in best.values():
                waits.append((spos, producer_idx(w), w))
        if not has_multi:
            continue
        bidx_of = {spos: stream[spos] for spos in range(len(stream))}
        free = sorted(movable_spos)
        assign = {}
        for carrier, pbidx, w in sorted(waits, key=lambda t: (t[0], -t[1])):
            chosen = None
            for spos in reversed(free):
                if spos > carrier:
                    continue
                if bidx_of[spos] <= pbidx:
                    break
                chosen = spos
                break
            if chosen is None:
                stuck.append((insts[stream[carrier]].name,
                              insts[stream[carrier]].__class__.__name__,
                              w.ant_name, w.wait_value))
                continue
            free.remove(chosen)
            assign.setdefault(chosen, []).append(w)
        for spos in range(len(stream)):
            inst = insts[stream[spos]]
            si = inst.sync_info
            ups = list(si.on_update) if si is not None else []
            new_w = pinned.get(spos, []) + assign.get(spos, [])
            if si is None and not new_w:
                continue
            inst.sync_info = _mb.SyncInfo(on_wait=new_w, on_update=ups)
    if stuck:
        raise RuntimeError(f"wait legalization failed: {stuck[:8]}")


def _get_program():
    if "nc" not in _CACHE:
        _CACHE["nc"] = _build_program()
    return _CACHE["nc"]


def _prep_core_inputs(c, enc, maskf, hidden, coverage, attn_w, attn_b,
                      dec_w, dec_b, w_eff, cvg_b, v):
    s = slice(c * BLOC, (c + 1) * BLOC)
    enc_l = enc[s]                                   # [BLOC, L, H]
    return {
        "encT": np.ascontiguousarray(enc_l.transpose(0, 2, 1)),
        "attn_wT": np.ascontiguousarray(attn_w.T),
        "w_effT": np.ascontiguousarray(w_eff.T),
        "dec_wT": np.ascontiguousarray(dec_w.T),
        "vT": np.ascontiguousarray(v[s].T),
        "covT": np.ascontiguousarray(coverage[s].T),
        "hidT": np.ascontiguousarray(hidden[s].T),
        "cov_in": np.ascontiguousarray(coverage[s]),
        "maskb": np.ascontiguousarray(maskf[s]),
        "attn_b_r": np.ascontiguousarray(attn_b[None, :]),
        "dec_b_r": np.ascontiguousarray(dec_b[None, :]),
        "cvg_b_r": np.ascontiguousarray(cvg_b[None, :]),
    }


def kernel(encoder_outputs, attn_mask, hidden, coverage,
           attn_w, attn_b, dec_w, dec_b, cvg_w, cvg_b, v):
    enc = np.asarray(encoder_outputs, dtype=np.float32)
    mask = np.asarray(attn_mask)
    hidden = np.asarray(hidden, dtype=np.float32)
    coverage = np.asarray(coverage, dtype=np.float32)
    attn_w = np.asarray(attn_w, dtype=np.float32)
    attn_b = np.asarray(attn_b, dtype=np.float32)
    dec_w = np.asarray(dec_w, dtype=np.float32)
    dec_b = np.asarray(dec_b, dtype=np.float32)
    cvg_b = np.asarray(cvg_b, dtype=np.float32)
    v = np.asarray(v, dtype=np.float32)
    # 'same' padding with kernel (1, H) on a single pixel: only the center
    # column of the conv weight is ever active.
    center = (H - 1) // 2
    w_eff = np.asarray(cvg_w[:, :, 0, center], dtype=np.float32)
    maskf = np.where(mask == 1, np.float32(0.0), np.float32(-1e38))

    nc = _get_program()
    in_maps = [
        _prep_core_inputs(c, enc, maskf, hidden, coverage, attn_w, attn_b,
                          dec_w, dec_b, w_eff, cvg_b, v)
        for c in range(NCORES)
    ]
    trace = os.environ.get("KERNEL_TRACE", "") == "1"
    res = run_bass_kernel_spmd(nc, in_maps, core_ids=list(range(NCORES)),
                               trace=trace)
    if trace and res.exec_time_ns is not None:
        _CACHE["exec_time_ns"] = res.exec_time_ns
        _CACHE["mean_exec_time_ns"] = res.mean_exec_time_ns
        _CACHE["trace"] = res.instructions_and_trace

    ctx = np.empty((B, H), np.float32)
    aw = np.empty((B, L), np.float32)
    ncov = np.empty((B, L), np.float32)
    for c in range(NCORES):
        r = res.results[c]
        s = slice(c * BLOC, (c + 1) * BLOC)
        aw[s] = r["aw_out"]
        ncov[s] = r["ncov_out"]
        # ctxT_out[p, k, b] -> ctx[b, k*128+p]
        ctx[s] = r["ctxT_out"].transpose(2, 1, 0).reshape(BLOC, H)
    return ctx, aw, ncov


# revision 33
# speedup vs baseline: 1.4371x; 1.0072x over previous
"""Trainium2 Bass kernel for nn_AttnCalc (coverage attention).

Contract: kernel(**inputs) takes FULL unsharded numpy inputs, distributes
batch-parallel across 8 NeuronCores, returns the full
(context_vector, attn_weights, new_coverage) tuple like the reference.

Math per batch b:
  enc_feat = enc[b] @ attn_w.T + attn_b          [L,H]
  dec_feat = dec_w @ hidden[b] + dec_b           [H]
  cov_feat = w_eff @ coverage[b] + cvg_b         [L]   (w_eff = cvg_w[:,:,0,(H-1)//2])
  feats    = tanh(enc_feat + dec_feat + cov_feat[:,None])
  scores   = feats @ v[b]  (masked, softmax over L) -> aw
  new_cov  = coverage[b] + aw
  context  = aw @ enc[b]                         [H]

Device layout (per core, BLOC=8 batches):
  [*,L]-major work uses tiles [128 (H-chunk), L] (transposed layout,
  contraction over H on PE partitions).  cov_feat and dec_feat(+biases)
  are folded into the enc_feat PSUM accumulation (K=1 rank-1 matmuls)
  and the tanh activation bias.  The context contraction (over L) runs
  on the Vector engine as tensor_tensor_reduce against a DMA-broadcast
  of aw, so encoder_outputs is only loaded once, in transposed form.

The target walrus build allows only ONE semaphore wait per TPB compute
instruction, so the program is arranged so every compute op has at most
one cross-engine dependency; tiny "absorber" ops (1x1 matmul / copy)
pick up the remaining waits ahead of real work.
"""

import sys
import os

sys.path.insert(0, "/opt/trn_rl_repo")

import numpy as np

import concourse.bass as bass
import concourse.tile as tile
from concourse import mybir
from concourse.bass_utils import run_bass_kernel_spmd
from concourse.tile_rust import add_dep_helper

B, L, H = 64, 512, 512
NCORES = 8
BLOC = B // NCORES          # batches per core
P = 128                     # SBUF partitions
PC = H // P                 # 128-chunks along H (== along L)
F32 = mybir.dt.float32
F32R = mybir.dt.float32r
Tanh = mybir.ActivationFunctionType.Tanh
Exp = mybir.ActivationFunctionType.Exp
Copy = mybir.ActivationFunctionType.Copy

_CACHE = {}


def _build_program():
    nc = bass.Bass()

    encT = nc.declare_dram_parameter("encT", [BLOC, H, L], F32R, isOutput=False)
    attn_wT = nc.declare_dram_parameter("attn_wT", [H, H], F32R, isOutput=False)
    w_effT = nc.declare_dram_parameter("w_effT", [L, L], F32, isOutput=False)
    dec_wT = nc.declare_dram_parameter("dec_wT", [H, H], F32, isOutput=False)
    vT = nc.declare_dram_parameter("vT", [H, BLOC], F32R, isOutput=False)
    covT = nc.declare_dram_parameter("covT", [L, BLOC], F32, isOutput=False)
    hidT = nc.declare_dram_parameter("hidT", [H, BLOC], F32, isOutput=False)
    cov_in = nc.declare_dram_parameter("cov_in", [BLOC, L], F32, isOutput=False)
    maskb = nc.declare_dram_parameter("maskb", [BLOC, L], F32, isOutput=False)
    attn_b_r = nc.declare_dram_parameter("attn_b_r", [1, H], F32, isOutput=False)
    dec_b_r = nc.declare_dram_parameter("dec_b_r", [1, H], F32, isOutput=False)
    cvg_b_r = nc.declare_dram_parameter("cvg_b_r", [1, L], F32, isOutput=False)

    aw_out = nc.declare_dram_parameter("aw_out", [BLOC, L], F32, isOutput=True)
    ncov_out = nc.declare_dram_parameter("ncov_out", [BLOC, L], F32, isOutput=True)
    ctxT_out = nc.declare_dram_parameter("ctxT_out", [P, PC, BLOC], F32, isOutput=True)

    def row3(dram2d):
        # [BLOC, L] dram -> [1, BLOC, L] AP so rows can live on partition 0
        return dram2d[:, :].rearrange("b l -> (b l)")[None].rearrange(
            "o (b l) -> o b l", b=BLOC)

    with tile.TileContext(nc) as tc:
        with (
            tc.tile_pool(name="const", bufs=1) as const,
            tc.tile_pool(name="enc", bufs=3) as epool,
            tc.tile_pool(name="feat", bufs=3) as fpool,
            tc.tile_pool(name="awb", bufs=3) as bpool,
            tc.tile_pool(name="prod", bufs=2) as prpool,
            tc.tile_pool(name="eps", bufs=4, space=bass.MemorySpace.PSUM) as ppool,
            tc.tile_pool(name="scps", bufs=2, space=bass.MemorySpace.PSUM) as scpool,
            tc.tile_pool(name="setps", bufs=1, space=bass.MemorySpace.PSUM) as stpool,
            tc.tile_pool(name="dumps", bufs=1, space=bass.MemorySpace.PSUM) as dumpool,
            tc.tile_pool(name="dram", bufs=1, space=bass.MemorySpace.DRAM) as dpool,
        ):
            # -------- wait absorbers (1x1 ops that pick up semaphore waits
            # so real compute ops never need more than one) --------
            dum_t = dumpool.tile([1, 64], F32, tag="dummy")
            dve_dum = const.tile([1, 256], F32)
            act_dum = const.tile([1, 256], F32)
            _ctr = {"pe": 0, "dve": 0, "act": 0}

            def pe_abs(ap):
                i = _ctr["pe"] = (_ctr["pe"] + 1) % 64
                ap = ap.bitcast(F32)
                return nc.tensor.matmul(dum_t[0:1, i:i + 1], ap, ap,
                                        start=True, stop=True)

            def dve_abs(ap):
                i = _ctr["dve"] = (_ctr["dve"] + 1) % 256
                return nc.vector.tensor_copy(dve_dum[0:1, i:i + 1], ap)

            def act_abs(ap):
                i = _ctr["act"] = (_ctr["act"] + 1) % 256
                return nc.scalar.activation(act_dum[0:1, i:i + 1], ap, Copy)

            def pin(real, *deps):
                for d in deps:
                    add_dep_helper(real.ins, d.ins, sync=False,
                                   reason="absorber ordering")

            # ---------------- constants ----------------
            wA = const.tile([P, PC, H], F32R)  # attn_wT  [h=k*128+p][o]
            wC = const.tile([P, PC, L], F32)   # w_effT   [l'=k*128+p][l]
            wD = const.tile([P, PC, H], F32)   # dec_wT   [h=k*128+p][o]
            vS = const.tile([P, PC, BLOC], F32R)
            cS = const.tile([P, PC, BLOC], F32)
            hS = const.tile([P, PC, BLOC], F32)
            nc.sync.dma_start(out=wA, in_=attn_wT[:, :].rearrange("(k p) o -> p k o", p=P))
            nc.sync.dma_start(out=wC, in_=w_effT[:, :].rearrange("(k p) o -> p k o", p=P))
            nc.sync.dma_start(out=wD, in_=dec_wT[:, :].rearrange("(k p) o -> p k o", p=P))
            nc.sync.dma_start(out=vS, in_=vT[:, :].rearrange("(k p) b -> p k b", p=P))
            nc.sync.dma_start(out=cS, in_=covT[:, :].rearrange("(k p) b -> p k b", p=P))
            nc.sync.dma_start(out=hS, in_=hidT[:, :].rearrange("(k p) b -> p k b", p=P))

            ab_row = const.tile([1, H], F32)
            db_row = const.tile([1, H], F32)
            cb_row = const.tile([1, L], F32)
            nc.sync.dma_start(out=ab_row, in_=attn_b_r[:, :])
            nc.sync.dma_start(out=db_row, in_=dec_b_r[:, :])
            nc.sync.dma_start(out=cb_row, in_=cvg_b_r[:, :])

            ones_b = const.tile([1, BLOC], F32)
            nc.vector.memset(ones_b, 1.0)
            ones_p = const.tile([1, P], F32)
            nc.vector.memset(ones_p, 1.0)
            d_ab = dve_abs(ab_row[0:1, 0:1])
            d_db = dve_abs(db_row[0:1, 0:1])
            brow = const.tile([1, H], F32)     # attn_b + dec_b
            badd = nc.vector.tensor_add(brow, ab_row, db_row)
            pin(badd, d_ab, d_db)

            # per-batch rows as disjoint slices of persistent [1,BLOC,L]
            # tiles on partition 0 (no slot reuse -> no release waits)
            mb = const.tile([1, BLOC, L], F32)
            covin = const.tile([1, BLOC, L], F32)
            nc.sync.dma_start(out=mb, in_=row3(maskb))
            nc.sync.dma_start(out=covin, in_=row3(cov_in))
            d_mb = dve_abs(mb[0:1, 0, 0:1])
            d_cvn = dve_abs(covin[0:1, 0, 0:1])

            sc = const.tile([1, BLOC, L], F32)      # scores -> exp -> aw
            nmx = const.tile([1, BLOC, 1], F32)
            se = const.tile([1, BLOC, 1], F32)
            rse = const.tile([1, BLOC, 1], F32)
            cov_sb = const.tile([1, BLOC, L], F32)  # cov_feat rows
            ctx_all = const.tile([P, PC, BLOC], F32)

            # ---------------- setup: cov_feat rows ----------------
            d_wC = pe_abs(wC[0:1, 0, 0:1])
            d_cS = pe_abs(cS[0:1, 0, 0:1])
            d_cb = pe_abs(cb_row[0:1, 0:1])
            d_on = pe_abs(ones_b[0:1, 0:1])
            cov_ps = stpool.tile([BLOC, L], F32, tag="setup")
            for k in range(PC):
                mm = nc.tensor.matmul(cov_ps, cS[:, k, :], wC[:, k, :],
                                      start=(k == 0), stop=False)
                if k == 0:
                    pin(mm, d_wC, d_cS)
            mm = nc.tensor.matmul(cov_ps, ones_b[:, :], cb_row[:, :],
                                  start=False, stop=True)
            pin(mm, d_cb, d_on)
            # bounce through DRAM to turn [BLOC(part), L] into row-major
            # [1, BLOC, L] on partition 0
            cov_tmp = const.tile([BLOC, L], F32)
            nc.vector.tensor_copy(cov_tmp, cov_ps)
            cov_dram = dpool.tile([BLOC, L], F32)
            nc.sync.dma_start(out=cov_dram[:, :], in_=cov_tmp)
            nc.sync.dma_start(out=cov_sb, in_=row3(cov_dram))
            d_cov = pe_abs(cov_sb[0:1, 0, 0:1])

            # ---------------- setup: tanh bias rows ----------------
            # bias_sb[p, o_chunk, b] = (dec_w @ hidden[b] + dec_b + attn_b)[o]
            d_wD = pe_abs(wD[0:1, 0, 0:1])
            d_hS = pe_abs(hS[0:1, 0, 0:1])
            d_br = pe_abs(brow[0:1, 0:1])
            dps = stpool.tile([P, PC, BLOC], F32, tag="setup")
            for o in range(PC):
                for k in range(PC):
                    mm = nc.tensor.matmul(dps[:, o, :], wD[:, k, o * P:(o + 1) * P],
                                          hS[:, k, :], start=(k == 0), stop=False)
                    if k == 0:
                        pin(mm, d_wD, d_hS)
                bmm = nc.tensor.matmul(dps[:, o, :], brow[0:1, o * P:(o + 1) * P],
                                       ones_b[:, :], start=False, stop=True)
                pin(bmm, d_br)
            bias_sb = const.tile([P, PC, BLOC], F32)
            nc.scalar.copy(bias_sb, dps)
            a_bias = act_abs(bias_sb[0:1, 0, 0:1])
            d_vS = pe_abs(vS[0:1, 0, 0:1])
            d_wA = pe_abs(wA[0:1, 0, 0:1])
            d_op = pe_abs(ones_p[0:1, 0:1])

            # ---------------- main pipeline (software-pipelined) ----------
            # produce(b): load eT, enc matmuls, +cov, tanh -> ft
            # consume(b-1): scores, softmax, aw broadcast, ncov, context
            # Emitting produce(b) before consume(b-1) keeps PE streaming:
            # scores(b-1) deps (tanh b-1) complete while enc(b) runs.
            prev_exp = None
            prev_df = None
            state = {}
            for it in range(BLOC + 1):
                if it < BLOC:
                    b = it
                    sps = [nc.sync.nop(nofuse=True) for _ in range(3)]
                    if prev_df is not None:
                        pin(sps[0], prev_df)
                    pin(sps[1], sps[0])
                    pin(sps[2], sps[1])
                    eT = epool.tile([P, PC, L], F32R)
                    eT_dma = nc.sync.dma_start(
                        out=eT, in_=encT[b].rearrange("(k p) l -> p k l", p=P))
                    pin(eT_dma, sps[2])
                    d_e = pe_abs(eT[0:1, 0, 0:1])
                    v_e = dve_abs(eT[0:1, 0, 0:1])

                    gp_cv = [nc.gpsimd.nop(nofuse=True) for _ in range(2)]
                    pin(gp_cv[0], d_e)
                    pin(gp_cv[1], gp_cv[0])
                    cb_t = bpool.tile([P, L], F32, tag="covb")
                    cv_dma = nc.gpsimd.dma_start(
                        out=cb_t, in_=cov_dram[b:b + 1, :].to_broadcast([P, L]))
                    pin(cv_dma, gp_cv[1])
                    v_cb = dve_abs(cb_t[0:1, 0:1])

                    a_slot = act_abs(ones_b[0:1, 0:1])
                    a_slot2 = act_abs(ones_b[0:1, 0:1])
                    if prev_exp is not None:
                        pin(a_slot, prev_exp)
                    pin(a_slot2, a_slot)
                    ft = fpool.tile([P, PC, L], F32R)
                    first_th = None
                    for o in range(PC):
                        ps = ppool.tile([P, L], F32, tag="encps")
                        for k in range(PC):
                            mm = nc.tensor.matmul(ps, wA[:, k, o * P:(o + 1) * P],
                                                  eT[:, k, :], start=(k == 0),
                                                  stop=(k == 3))
                            if k == 0:
                                pin(mm, d_e)
                                if b == 0:
                                    pin(mm, d_wA)
                        ta = nc.vector.tensor_add(ft[:, o, :], ps, cb_t)
                        if o == 0:
                            pin(ta, v_cb)
                        th = nc.scalar.activation(
                            out=ft[:, o, :], in_=ft[:, o, :], func=Tanh,
                            bias=bias_sb[:, o, b:b + 1], scale=1.0)
                        if first_th is None:
                            first_th = th
                            pin(th, a_slot2)
                        if b == 0 and o == 0:
                            pin(th, a_bias)
                    state[b] = (eT, ft, v_e)

                if it >= 1:
                    b = it - 1
                    eT, ft, v_e = state.pop(b)
                    # scores[l] = sum_o feats[o,l] * v[b,o]
                    d_f = pe_abs(ft[0:1, 0, 0:1].bitcast(F32))
                    prev_df = d_f
                    sc_ps = scpool.tile([1, L], F32)
                    for k in range(PC):
                        mm = nc.tensor.matmul(sc_ps, vS[:, k, b:b + 1],
                                              ft[:, k, :],
                                              start=(k == 0), stop=(k == 3))
                        if k == 0:
                            pin(mm, d_f)
                            if b == 0:
                                pin(mm, d_vS)

                    # masked softmax over l, in sc row b (aw ends up there)
                    scr = sc[0:1, b, :]
                    madd = nc.vector.tensor_add(scr, sc_ps, mb[0:1, b, :])
                    if b == 0:
                        pin(madd, d_mb)
                    nc.vector.tensor_reduce(out=nmx[0:1, b, :], in_=scr,
                                            axis=mybir.AxisListType.X,
                                            op=mybir.AluOpType.max, negate=True)
                    prev_exp = nc.scalar.activation(
                        out=scr, in_=scr, func=Exp,
                        bias=nmx[0:1, b, :], scale=1.0,
                        accum_out=se[0:1, b, :])
                    nc.vector.reciprocal(rse[0:1, b, :], se[0:1, b, :])
                    nc.vector.tensor_scalar_mul(scr, scr, rse[0:1, b, :])

                    # aw row out + broadcast back for the context reduction
                    sp_aw = nc.sync.nop(nofuse=True)
                    pin(sp_aw, mm)
                    aw_dma = nc.sync.dma_start(out=aw_out[b:b + 1, :], in_=scr)
                    pin(aw_dma, sp_aw)
                    gp_slots = [nc.gpsimd.nop(nofuse=True) for _ in range(2)]
                    pin(gp_slots[0], mm)
                    pin(gp_slots[1], gp_slots[0])
                    aw_b = bpool.tile([P, L], F32, tag="awb")
                    bc_dma = nc.gpsimd.dma_start(
                        out=aw_b, in_=aw_out[b:b + 1, :].to_broadcast([P, L]))
                    pin(bc_dma, gp_slots[1])

                    # new_coverage row (in place over covin row)
                    ncadd = nc.vector.tensor_add(covin[0:1, b, :],
                                                  covin[0:1, b, :], scr)
                    if b == 0:
                        pin(ncadd, d_cvn)
                    sp_nc = nc.sync.nop(nofuse=True)
                    pin(sp_nc, aw_dma)
                    nc_dma = nc.sync.dma_start(out=ncov_out[b:b + 1, :],
                                               in_=covin[0:1, b, :])
                    pin(nc_dma, sp_nc)

                    # context[h] = sum_l aw[l] * encT[h,l] on DVE
                    for o in range(PC):
                        prod = prpool.tile([P, L], F32, tag="prod")
                        tm = nc.vector.tensor_mul(prod, eT[:, o, :].bitcast(F32),
                                                   aw_b)
                        if o == 0:
                            pin(tm, v_e)
                        nc.vector.reduce_sum(out=ctx_all[:, o, b:b + 1],
                                             in_=prod,
                                             axis=mybir.AxisListType.X)

            ctx_dma = nc.sync.dma_start(out=ctxT_out[:, :, :], in_=ctx_all)

            # tail landing slots: the framework kernel-tail drain waits on
            # every engine/queue semaphore at once; give the legalizer SP
            # instructions to spread those waits over
            tail = ctx_dma
            for _ in range(22):
                n = nc.sync.nop(nofuse=True)
                pin(n, tail)
                tail = n

    _legalize_waits(nc)
    return nc


# The nix walrus build (setupSyncWait) accepts only ONE sync wait per TPB
# instruction (compute and DMA alike).  Tile can emit several.  Because the
# committed instruction order is a topological order of the dependency
# graph, a wait whose producing semaphore update completes at block index p
# can be safely carried by ANY same-engine instruction at index > p that
# precedes the original carrier: engines execute in order, so the original
# instruction still starts after the wait is satisfied, and the producer
# (committed before the new carrier) cannot depend on it -- no deadlock.
# Assign waits to instructions as an interval matching problem.
def _legalize_waits(nc):
    import concourse.mybir as _mb

    fn = nc.m.functions[0]
    stuck = []
    NO_LANDING = ("InstISA", "InstEventSemaphore", "InstUnconditionalBranch",
                  "InstCall", "InstRegisterMove", "InstHalt")
    # one global stream per engine, blocks concatenated in order (engines
    # branch from block to block in order, so per-engine execution order is
    # block order)
    insts = []
    for blk in fn.blocks:
        insts.extend(blk.instructions)

    sem_hist = {}
    cum = {}
    streams = {}
    for i, inst in enumerate(insts):
        si = inst.sync_info
        if si is not None:
            for u in si.on_update:
                cum[u.id] = cum.get(u.id, 0) + u.update_value
                sem_hist.setdefault(u.id, []).append((i, cum[u.id]))
        streams.setdefault(inst.engine, []).append(i)

    def producer_idx(w):
        hist = sem_hist.get(w.id)
        if hist is None:
            return None            # unknown semaphore: not movable
        for i, v in hist:
            if v >= w.wait_value:
                return i
        return None

    for eng, stream in streams.items():
        movable_spos = []
        pinned = {}                # spos -> unmovable waits
        waits = []                 # (carrier_spos, producer_bidx, wait)
        has_multi = False
        for spos, i in enumerate(stream):
            inst = insts[i]
            si = inst.sync_info
            ws = list(si.on_wait) if si is not None else []
            if len(ws) > 1:
                has_multi = True

            def mov(w):
                if w.wait_reg is not None or w.wait_value <= 0:
                    return False
                p = producer_idx(w)
                return p is not None and p < i
            special = inst.__class__.__name__ in NO_LANDING
            unmov = [w for w in ws if special or not mov(w)]
            if unmov:
                pinned[spos] = unmov
            elif not special:
                movable_spos.append(spos)
            if special:
                continue
            best = {}
            for w in ws:
                if not mov(w):
                    continue
                if w.id not in best or w.wait_value > best[w.id].wait_value:
                    best[w.id] = w
            for w in best.values():
                waits.append((spos, producer_idx(w), w))
        if not has_multi:
            continue
        bidx_of = {spos: stream[spos] for spos in range(len(stream))}
        free = sorted(movable_spos)
        assign = {}
        for carrier, pbidx, w in sorted(waits, key=lambda t: (t[0], -t[1])):
            chosen = None
            for spos in reversed(free):
                if spos > carrier:
                    continue
                if bidx_of[spos] <= pbidx:
                    break
                chosen = spos
                break
            if chosen is None:
                stuck.append((insts[stream[carrier]].name,
                              insts[stream[carrier]].__class__.__name__,
                              w.ant_name, w.wait_value))
                continue
            free.remove(chosen)
            assign.setdefault(chosen, []).append(w)
        for spos in range(len(stream)):
            inst = insts[stream[spos]]
            si = inst.sync_info
            ups = list(si.on_update) if si is not None else []
            new_w = pinned.get(spos, []) + assign.get(spos, [])
            if si is None and not new_w:
                continue
            inst.sync_info = _mb.SyncInfo(on_wait=new_w, on_update=ups)
    if stuck:
        raise RuntimeError(f"wait legalization failed: {stuck[:8]}")


def _get_program():
    if "nc" not in _CACHE:
        _CACHE["nc"] = _build_program()
    return _CACHE["nc"]


def _prep_core_inputs(c, enc, maskf, hidden, coverage, attn_w, attn_b,
                      dec_w, dec_b, w_eff, cvg_b, v):
    s = slice(c * BLOC, (c + 1) * BLOC)
    enc_l = enc[s]                                   # [BLOC, L, H]
    return {
        "encT": np.ascontiguousarray(enc_l.transpose(0, 2, 1)),
        "attn_wT": np.ascontiguousarray(attn_w.T),
        "w_effT": np.ascontiguousarray(w_eff.T),
        "dec_wT": np.ascontiguousarray(dec_w.T),
        "vT": np.ascontiguousarray(v[s].T),
        "covT": np.ascontiguousarray(coverage[s].T),
        "hidT": np.ascontiguousarray(hidden[s].T),
        "cov_in": np.ascontiguousarray(coverage[s]),
        "maskb": np.ascontiguousarray(maskf[s]),
        "attn_b_r": np.ascontiguousarray(attn_b[None, :]),
        "dec_b_r": np.ascontiguousarray(dec_b[None, :]),
        "cvg_b_r": np.ascontiguousarray(cvg_b[None, :]),
    }


def kernel(encoder_outputs, attn_mask, hidden, coverage,
           attn_w, attn_b, dec_w, dec_b, cvg_w, cvg_b, v):
    enc = np.asarray(encoder_outputs, dtype=np.float32)
    mask = np.asarray(attn_mask)
    hidden = np.asarray(hidden, dtype=np.float32)
    coverage = np.asarray(coverage, dtype=np.float32)
    attn_w = np.asarray(attn_w, dtype=np.float32)
    attn_b = np.asarray(attn_b, dtype=np.float32)
    dec_w = np.asarray(dec_w, dtype=np.float32)
    dec_b = np.asarray(dec_b, dtype=np.float32)
    cvg_b = np.asarray(cvg_b, dtype=np.float32)
    v = np.asarray(v, dtype=np.float32)
    # 'same' padding with kernel (1, H) on a single pixel: only the center
    # column of the conv weight is ever active.
    center = (H - 1) // 2
    w_eff = np.asarray(cvg_w[:, :, 0, center], dtype=np.float32)
    maskf = np.where(mask == 1, np.float32(0.0), np.float32(-1e38))

    nc = _get_program()
    in_maps = [
        _prep_core_inputs(c, enc, maskf, hidden, coverage, attn_w, attn_b,
                          dec_w, dec_b, w_eff, cvg_b, v)
        for c in range(NCORES)
    ]
    trace = os.environ.get("KERNEL_TRACE", "") == "1"
    res = run_bass_kernel_spmd(nc, in_maps, core_ids=list(range(NCORES)),
                               trace=trace)
    if trace and res.exec_time_ns is not None:
        _CACHE["exec_time_ns"] = res.exec_time_ns
        _CACHE["mean_exec_time_ns"] = res.mean_exec_time_ns
        _CACHE["trace"] = res.instructions_and_trace

    ctx = np.empty((B, H), np.float32)
    aw = np.empty((B, L), np.float32)
    ncov = np.empty((B, L), np.float32)
    for c in range(NCORES):
        r = res.results[c]
        s = slice(c * BLOC, (c + 1) * BLOC)
        aw[s] = r["aw_out"]
        ncov[s] = r["ncov_out"]
        # ctxT_out[p, k, b] -> ctx[b, k*128+p]
        ctx[s] = r["ctxT_out"].transpose(2, 1, 0).reshape(BLOC, H)
    return ctx, aw, ncov
